# revision 15
# baseline (speedup 1.0000x reference)
"""Trainium2 Bass kernel for nn_ConvModule: LN -> 1x1 conv (D->2I) -> SwiGLU
-> depthwise conv (K=31) -> PReLU -> 1x1 conv (I->D).

Sharding: data-parallel over batch, 2 batches per core across 8 cores.

v3 design:
  - Host-side weight preprocessing (w1^T*ln_g bf16, b1'=b1+w1@ln_b, w2^T bf16,
    conv tap diagonals bf16, per-channel tap/alpha/dwb columns).
  - bf16 on-chip data path (DVE 2x/4x perf modes; PE bf16 matmuls).
  - LN stats via ACT accum passes; normalize on ACT (scale/bias [P,1] APs);
    xn^T via PE transposes (bf16) + ACT PSUM->SBUF copies.
  - Depthwise conv split: cbs 0-3 on the PE (31 diagonal matmuls per panel);
    cbs 4-7 on the DVE as product(tensor_scalar 4x) + add(tensor_tensor 2x)
    trees over full strips, with an odd-shifted strip copy so every window
    is 4B-aligned. scalar_tensor_tensor is avoided (no 2x uop).
  - PReLU on ACT (Prelu w/ per-channel alpha AP; dwb fused as bias for PE
    panels, pre-added via the product seed for DVE chains).
  - GEMM2 with v tiles stationary; b2 via a K=1 ones-row matmul.
"""

import sys

sys.path.insert(0, "/opt/trn_rl_repo")

from contextlib import ExitStack

import numpy as np
import ml_dtypes

import concourse.bacc as bacc
import concourse.tile as tile
from concourse import mybir
from concourse.masks import make_identity
from concourse.bass_utils import run_bass_kernel_spmd

B, T, D, I, K = 16, 2048, 512, 1024, 31
NCORES = 8
BPC = B // NCORES  # batches per core
E = 2 * I  # 2048
TP = T // 512  # time panels per batch (4)
CB = I // 128  # channel blocks (8)
DCH = D // 128  # d chunks (4)
PADL = 16  # left pad (even so DVE chain windows stay 4B-aligned)
STRIPW = PADL + T + 16  # 2080
P = 128

F32 = mybir.dt.float32
BF16 = mybir.dt.bfloat16
ALU = mybir.AluOpType
ACTF = mybir.ActivationFunctionType

# cbs 0..PE_NCB-1 on the PE, the rest on DVE chains
PE_NCB = 4
PE_CBS = list(range(PE_NCB))
PE_CB_IDX = {cb: i for i, cb in enumerate(PE_CBS)}
DVE_CBS = list(range(PE_NCB, CB))


def _build_kernel(ctx, tc):
    nc = tc.nc
    x_d = nc.dram_tensor("x", [BPC, T, D], F32, kind="ExternalInput").ap()
    w1p_d = nc.dram_tensor("w1p", [D, E], BF16, kind="ExternalInput").ap()
    b1p_d = nc.dram_tensor("b1p", [E], F32, kind="ExternalInput").ap()
    w2t_d = nc.dram_tensor("w2t", [I, D], BF16, kind="ExternalInput").ap()
    diag_d = nc.dram_tensor(
        "diag", [P, len(PE_CBS) * K * P], BF16, kind="ExternalInput").ap()
    dwc_d = nc.dram_tensor("dwc", [P, CB * K], F32, kind="ExternalInput").ap()
    dwb_d = nc.dram_tensor("dwbc", [P, CB], F32, kind="ExternalInput").ap()
    alpha_d = nc.dram_tensor("alphac", [P, CB], F32, kind="ExternalInput").ap()
    b2r_d = nc.dram_tensor("b2r", [1, D], BF16, kind="ExternalInput").ap()
    ones_d = nc.dram_tensor("ones", [1, P], BF16, kind="ExternalInput").ap()
    out_d = nc.dram_tensor("out", [BPC, T, D], F32, kind="ExternalOutput").ap()

    const = ctx.enter_context(tc.tile_pool(name="const", bufs=1))
    psum = ctx.enter_context(tc.tile_pool(name="psum", bufs=6, space="PSUM"))

    ident_bf = const.tile([P, P], BF16, tag="ident_bf")
    make_identity(nc, ident_bf[:])

    # ---- constants / weights (all host-prepared) ----
    w1t = [const.tile([P, E], BF16, tag=f"w1t{j}", name=f"w1t{j}")
           for j in range(DCH)]
    for j in range(DCH):
        nc.sync.dma_start(w1t[j][:], w1p_d[j * P:(j + 1) * P, :])
    w2t = [const.tile([P, D], BF16, tag=f"w2t{i}", name=f"w2t{i}")
           for i in range(CB)]
    for i in range(CB):
        nc.sync.dma_start(w2t[i][:], w2t_d[i * P:(i + 1) * P, :])
    b1p = const.tile([P, 2 * CB], F32, tag="b1p")
    nc.sync.dma_start(b1p[:], b1p_d.rearrange("(i p) -> p i", p=P))
    dw_sb = const.tile([P, CB * K], F32, tag="dw_sb")
    nc.sync.dma_start(dw_sb[:], dwc_d)
    dwb_sb = const.tile([P, CB], F32, tag="dwb_sb")
    nc.sync.dma_start(dwb_sb[:], dwb_d)
    alpha_sb = const.tile([P, CB], F32, tag="alpha_sb")
    nc.sync.dma_start(alpha_sb[:], alpha_d)
    b2row = const.tile([1, D], BF16, tag="b2row")
    nc.sync.dma_start(b2row[:], b2r_d)
    ones_bf = const.tile([1, P], BF16, tag="ones_bf")
    nc.sync.dma_start(ones_bf[:], ones_d)
    eps_t = const.tile([P, 1], F32, tag="eps_t")
    nc.vector.memset(eps_t[:], 1e-5)
    diag_sb = const.tile([P, len(PE_CBS) * K * P], BF16, tag="diag")
    nc.sync.dma_start(diag_sb[:], diag_d)

    # ---- pools ----
    xpool = ctx.enter_context(tc.tile_pool(name="xpool", bufs=2))
    scr = ctx.enter_context(tc.tile_pool(name="scr", bufs=2))
    stat = ctx.enter_context(tc.tile_pool(name="stat", bufs=8))
    xnp = ctx.enter_context(tc.tile_pool(name="xnp", bufs=4))
    xntp = ctx.enter_context(tc.tile_pool(name="xntp", bufs=6))
    sw = ctx.enter_context(tc.tile_pool(name="sw", bufs=4))
    strips = ctx.enter_context(tc.tile_pool(name="strips", bufs=8))
    soddp = ctx.enter_context(tc.tile_pool(name="soddp", bufs=5))
    accp = ctx.enter_context(tc.tile_pool(name="accp", bufs=5))
    prodp = ctx.enter_context(tc.tile_pool(name="prodp", bufs=2))
    vtp = ctx.enter_context(tc.tile_pool(name="vtp", bufs=8))
    outp = ctx.enter_context(tc.tile_pool(name="outp", bufs=2))

    strip = {}
    sodd = {}
    vt = {}

    def load_x_panel(b, tp):
        tiles = []
        for tt in range(4):
            t0 = tp * 512 + tt * P
            x_t = xpool.tile([P, D], F32, tag="x", bufs=6,
                             name=f"x_{b}_{tp}_{tt}")
            nc.sync.dma_start(x_t[:], x_d[b, t0:t0 + P, :])
            tiles.append(x_t)
        return tiles

    xq = {}

    def emit_A_panel(b, tp):
        """LN stats + normalize + transpose + GEMM1 + SwiGLU for one panel."""
        if tp + 1 < TP:
            xq[(b, tp + 1)] = load_x_panel(b, tp + 1)
        elif b + 1 < BPC:
            xq[(b + 1, 0)] = load_x_panel(b + 1, 0)
        x_tiles = xq.pop((b, tp))

        ssum4 = stat.tile([P, 4], F32, tag="ssum4")
        ssq4 = stat.tile([P, 4], F32, tag="ssq4")
        for tt in range(4):
            xcp = scr.tile([P, D], BF16, tag="xscr", bufs=2)
            nc.scalar.activation(xcp[:], x_tiles[tt][:], ACTF.Identity,
                                 accum_out=ssum4[:, tt:tt + 1])
            xsq = scr.tile([P, D], BF16, tag="xscr", bufs=2)
            nc.scalar.activation(xsq[:], x_tiles[tt][:], ACTF.Square,
                                 accum_out=ssq4[:, tt:tt + 1])
        negmean4 = stat.tile([P, 4], F32, tag="negmean4")
        nc.vector.tensor_scalar_mul(negmean4[:], ssum4[:], -1.0 / D)
        ex24 = stat.tile([P, 4], F32, tag="ex24")
        nc.vector.tensor_scalar_mul(ex24[:], ssq4[:], 1.0 / D)
        mu24 = stat.tile([P, 4], F32, tag="mu24")
        nc.vector.tensor_mul(mu24[:], negmean4[:], negmean4[:])
        negv4 = stat.tile([P, 4], F32, tag="negv4")
        nc.vector.tensor_sub(negv4[:], mu24[:], ex24[:])
        stdv4 = stat.tile([P, 4], F32, tag="stdv4")
        nc.scalar.activation(stdv4[:], negv4[:], ACTF.Sqrt,
                             scale=-1.0, bias=eps_t[:])
        rstd4 = stat.tile([P, 4], F32, tag="rstd4")
        nc.vector.reciprocal(rstd4[:], stdv4[:])
        negmr4 = stat.tile([P, 4], F32, tag="negmr4")
        nc.vector.tensor_mul(negmr4[:], negmean4[:], rstd4[:])

        xn_tiles = []
        for tt in range(4):
            xn_t = xnp.tile([P, D], BF16, tag="xn")
            nc.scalar.activation(xn_t[:], x_tiles[tt][:], ACTF.Identity,
                                 bias=negmr4[:, tt:tt + 1],
                                 scale=rstd4[:, tt:tt + 1])
            xn_tiles.append(xn_t)
        xnt_p = []
        for j in range(DCH):
            ptr = psum.tile([P, 512], BF16, tag="pst", bufs=2)
            for tt in range(4):
                nc.tensor.transpose(ptr[:, tt * P:(tt + 1) * P],
                                    xn_tiles[tt][:, j * P:(j + 1) * P],
                                    ident_bf[:])
            xt = xntp.tile([P, 512], BF16, tag="xnt", name=f"xnt_{b}_{tp}_{j}")
            nc.scalar.activation(xt[:], ptr[:], ACTF.Copy)
            xnt_p.append(xt)

        for i in range(CB):
            ps_a = psum.tile([P, 512], F32, tag="ps")
            ps_g = psum.tile([P, 512], F32, tag="ps")
            for j in range(DCH):
                nc.tensor.matmul(
                    ps_a[:], w1t[j][:, i * P:(i + 1) * P], xnt_p[j][:],
                    start=(j == 0), stop=(j == DCH - 1))
            for j in range(DCH):
                nc.tensor.matmul(
                    ps_g[:], w1t[j][:, I + i * P:I + (i + 1) * P], xnt_p[j][:],
                    start=(j == 0), stop=(j == DCH - 1))
            s_sb = sw.tile([P, 512], BF16, tag="s_sb")
            nc.scalar.activation(s_sb[:], ps_g[:], ACTF.Silu,
                                 bias=b1p[:, CB + i:CB + i + 1])
            a_sb = sw.tile([P, 512], BF16, tag="a_sb")
            nc.scalar.activation(a_sb[:], ps_a[:], ACTF.Identity,
                                 bias=b1p[:, i:i + 1])
            nc.vector.tensor_mul(
                strip[(b, i)][:, PADL + tp * 512:PADL + (tp + 1) * 512],
                a_sb[:], s_sb[:])

    def alloc_strips(b):
        for cb in range(CB):
            s = strips.tile([P, STRIPW], BF16, tag="strip",
                            name=f"strip_{b}_{cb}")
            nc.gpsimd.memset(s[:, 0:PADL], 0.0)
            nc.gpsimd.memset(s[:, PADL + T:STRIPW], 0.0)
            strip[(b, cb)] = s
            vt[(b, cb)] = vtp.tile([P, T], BF16, tag="vt",
                                   name=f"vt_{b}_{cb}")

    def emit_odd_copy(b, cb):
        so = soddp.tile([P, STRIPW], BF16, tag="sodd", name=f"sodd_{b}_{cb}")
        nc.gpsimd.tensor_copy(so[:, 0:STRIPW - 2],
                              strip[(b, cb)][:, 1:STRIPW - 1])
        sodd[(b, cb)] = so

    def emit_conv_pe(b, cb, tp):
        ps_c = psum.tile([P, 512], F32, tag="ps")
        ci = PE_CB_IDX[cb]
        st = strip[(b, cb)]
        for k in range(K):
            nc.tensor.matmul(
                ps_c[:], diag_sb[:, (ci * K + k) * P:(ci * K + k + 1) * P],
                st[:, tp * 512 + k + 1:tp * 512 + k + 1 + 512],
                start=(k == 0), stop=(k == K - 1))
        nc.scalar.activation(vt[(b, cb)][:, tp * 512:(tp + 1) * 512], ps_c[:],
                             ACTF.Prelu, bias=dwb_sb[:, cb:cb + 1],
                             alpha=alpha_sb[:, cb:cb + 1])

    def win(b, cb, t0, k, L):
        off = t0 + k + 1
        if off % 2 == 0:
            return strip[(b, cb)][:, off:off + L]
        return sodd[(b, cb)][:, off - 1:off - 1 + L]

    def emit_chain(b, cb, t0, L, acc=None, helped=False):
        """product+add tree for strip cols [t0, t0+L); products optionally on
        gpsimd (helped=True), adds always on DVE. Returns the acc tile whose
        [t0-t0base...] region holds w; acc tiles are [P, T] indexed by t0."""
        peng = nc.gpsimd if helped else nc.vector
        if acc is None:
            acc = accp.tile([P, T], BF16, tag="acc", name=f"acc_{b}_{cb}")
        wcol = lambda k: dw_sb[:, cb * K + k:cb * K + k + 1]
        a = acc[:, t0:t0 + L]
        peng.tensor_scalar(a, win(b, cb, t0, 0, L), wcol(0),
                           dwb_sb[:, cb:cb + 1], op0=ALU.mult, op1=ALU.add)
        for k in range(1, K):
            pk = prodp.tile([P, 1024], BF16, tag="pk", bufs=3)
            peng.tensor_scalar_mul(pk[:, 0:L], win(b, cb, t0, k, L), wcol(k))
            nc.vector.tensor_add(a, a, pk[:, 0:L])
        return acc

    def emit_chain_prelu(b, cb, t0, L, acc):
        nc.scalar.activation(vt[(b, cb)][:, t0:t0 + L], acc[:, t0:t0 + L],
                             ACTF.Prelu, alpha=alpha_sb[:, cb:cb + 1])

    def emit_C(b, tp):
        for tt in range(4):
            ps_o = psum.tile([P, D], F32, tag="ps")
            nc.tensor.matmul(ps_o[:], ones_bf[:], b2row[:],
                             start=True, stop=False)
            c0 = tp * 512 + tt * P
            for cb in range(CB):
                nc.tensor.matmul(
                    ps_o[:], vt[(b, cb)][:, c0:c0 + P], w2t[cb][:],
                    start=False, stop=(cb == CB - 1))
            o_sb = outp.tile([P, D], F32, tag="o_sb")
            nc.scalar.activation(o_sb[:], ps_o[:], ACTF.Copy)
            nc.scalar.dma_start(out_d[b, c0:c0 + P, :], o_sb[:])

    # ================= emission =================
    xq[(0, 0)] = load_x_panel(0, 0)
    alloc_strips(0)
    for tp in range(TP):
        emit_A_panel(0, tp)
    for cb in range(3, CB):
        emit_odd_copy(0, cb)

    # conv b0: PE = cbs 0,1,2 full + cb3 tp0/tp1
    for cb in (0, 1, 2):
        for tp in range(TP):
            emit_conv_pe(0, cb, tp)
    emit_conv_pe(0, 3, 0)
    emit_conv_pe(0, 3, 1)

    # b0 chains (cb3 h2 + cb4..7), interleaved with A(1)
    acc0 = {}
    acc0[4] = emit_chain(0, 4, 0, 1024, helped=True)
    emit_chain(0, 4, 1024, 1024, acc=acc0[4], helped=True)
    acc0[5] = emit_chain(0, 5, 0, 1024, helped=True)
    alloc_strips(1)
    emit_A_panel(1, 0)
    emit_chain(0, 5, 1024, 1024, acc=acc0[5], helped=True)
    acc0[3] = emit_chain(0, 3, 1024, 1024, helped=False)
    emit_A_panel(1, 1)
    acc0[6] = emit_chain(0, 6, 0, 1024, helped=False)
    emit_A_panel(1, 2)
    emit_chain(0, 6, 1024, 1024, acc=acc0[6], helped=False)
    emit_A_panel(1, 3)
    acc0[7] = emit_chain(0, 7, 0, 1024, helped=False)
    emit_chain(0, 7, 1024, 1024, acc=acc0[7], helped=False)
    for cb in range(4, CB):
        emit_odd_copy(1, cb)
    emit_chain_prelu(0, 3, 1024, 1024, acc0[3])
    for cb in range(4, CB):
        emit_chain_prelu(0, cb, 0, T, acc0[cb])
    for tp in range(TP):
        emit_C(0, tp)

    # conv b1: PE = cbs 0..3 full; chains cb4..7 in column halves
    for cb in PE_CBS:
        for tp in range(TP):
            emit_conv_pe(1, cb, tp)
    acc1 = {}
    acc1[4] = emit_chain(1, 4, 0, 1024, helped=True)
    acc1[5] = emit_chain(1, 5, 0, 1024, helped=True)
    acc1[6] = emit_chain(1, 6, 0, 1024, helped=False)
    acc1[7] = emit_chain(1, 7, 0, 1024, helped=False)
    for cb in range(4, CB):
        emit_chain_prelu(1, cb, 0, 1024, acc1[cb])
    emit_C(1, 0)
    emit_C(1, 1)
    for cb in range(4, CB):
        emit_chain(1, cb, 1024, 1024, acc=acc1[cb], helped=(cb in (4, 5)))
    for cb in range(4, CB):
        emit_chain_prelu(1, cb, 1024, 1024, acc1[cb])
    emit_C(1, 2)
    emit_C(1, 3)


_NC_CACHE = None


def _get_program():
    global _NC_CACHE
    if _NC_CACHE is None:
        nc = bacc.Bacc("TRN2", target_bir_lowering=False, debug=False)
        with tile.TileContext(nc) as tc, ExitStack() as ctx:
            _build_kernel(ctx, tc)
        nc.compile()
        _NC_CACHE = nc
    return _NC_CACHE


def _host_prep(ln_g, ln_b, w1, b1, dw, dwb, alpha, w2, b2):
    bf = ml_dtypes.bfloat16
    w1 = np.asarray(w1, np.float32)
    ln_g = np.asarray(ln_g, np.float32)
    ln_b = np.asarray(ln_b, np.float32)
    dwf = np.asarray(dw, np.float32).reshape(I, K)
    w1p = np.ascontiguousarray((w1 * ln_g[None, :]).T).astype(bf)
    b1p = (np.asarray(b1, np.float32) + w1 @ ln_b).astype(np.float32)
    w2t = np.ascontiguousarray(np.asarray(w2, np.float32).T).astype(bf)
    diag = np.zeros((P, len(PE_CBS) * K * P), np.float32)
    ar = np.arange(P)
    for ci, cb in enumerate(PE_CBS):
        for k in range(K):
            diag[ar, (ci * K + k) * P + ar] = dwf[cb * P:(cb + 1) * P, k]
    diag = diag.astype(bf)
    dwc = np.ascontiguousarray(
        dwf.reshape(CB, P, K).transpose(1, 0, 2).reshape(P, CB * K)
    ).astype(np.float32)
    dwbc = np.ascontiguousarray(
        np.asarray(dwb, np.float32).reshape(CB, P).T).astype(np.float32)
    alphac = np.ascontiguousarray(
        np.asarray(alpha, np.float32).reshape(CB, P).T).astype(np.float32)
    b2r = np.asarray(b2, np.float32)[None, :].astype(bf)
    ones = np.ones((1, P), np.float32).astype(bf)
    return {"w1p": w1p, "b1p": b1p, "w2t": w2t, "diag": diag, "dwc": dwc,
            "dwbc": dwbc, "alphac": alphac, "b2r": b2r, "ones": ones}


def kernel(x, ln_g, ln_b, w1, b1, dw, dwb, alpha, w2, b2, _trace=False):
    nc = _get_program()
    x = np.ascontiguousarray(x, dtype=np.float32)
    shared = _host_prep(ln_g, ln_b, w1, b1, dw, dwb, alpha, w2, b2)
    in_maps = [
        {"x": x[c * BPC:(c + 1) * BPC], **shared} for c in range(NCORES)
    ]
    res = run_bass_kernel_spmd(nc, in_maps, core_ids=list(range(NCORES)),
                               trace=_trace)
    out = np.concatenate([res.results[c]["out"] for c in range(NCORES)], axis=0)
    if _trace:
        kernel.last_results = res
    return out


# revision 17
# speedup vs baseline: 6.7070x; 6.7070x over previous
"""Trainium2 Bass kernel for nn_ConvModule: LN -> 1x1 conv (D->2I) -> SwiGLU
-> depthwise conv (K=31) -> PReLU -> 1x1 conv (I->D).

Sharding: data-parallel over batch, 2 batches per core across 8 cores.

v3 design:
  - Host-side weight preprocessing (w1^T*ln_g bf16, b1'=b1+w1@ln_b, w2^T bf16,
    conv tap diagonals bf16, per-channel tap/alpha/dwb columns).
  - bf16 on-chip data path (DVE 2x/4x perf modes; PE bf16 matmuls).
  - LN stats via ACT accum passes; normalize on ACT (scale/bias [P,1] APs);
    xn^T via PE transposes (bf16) + ACT PSUM->SBUF copies.
  - Depthwise conv split: cbs 0-3 on the PE (31 diagonal matmuls per panel);
    cbs 4-7 on the DVE as product(tensor_scalar 4x) + add(tensor_tensor 2x)
    trees over full strips, with an odd-shifted strip copy so every window
    is 4B-aligned. scalar_tensor_tensor is avoided (no 2x uop).
  - PReLU on ACT (Prelu w/ per-channel alpha AP; dwb fused as bias for PE
    panels, pre-added via the product seed for DVE chains).
  - GEMM2 with v tiles stationary; b2 via a K=1 ones-row matmul.
"""

import sys

sys.path.insert(0, "/opt/trn_rl_repo")

from contextlib import ExitStack

import numpy as np
import ml_dtypes

import concourse.bacc as bacc
import concourse.tile as tile
from concourse import mybir
from concourse.masks import make_identity
from concourse.bass_utils import run_bass_kernel_spmd

B, T, D, I, K = 16, 2048, 512, 1024, 31
NCORES = 8
BPC = B // NCORES  # batches per core
E = 2 * I  # 2048
TP = T // 512  # time panels per batch (4)
CB = I // 128  # channel blocks (8)
DCH = D // 128  # d chunks (4)
PADL = 16  # left pad (even so DVE chain windows stay 4B-aligned)
STRIPW = PADL + T + 16  # 2080
P = 128

F32 = mybir.dt.float32
BF16 = mybir.dt.bfloat16
ALU = mybir.AluOpType
ACTF = mybir.ActivationFunctionType

# cbs 0..PE_NCB-1 on the PE, the rest on DVE chains
PE_NCB = 5
PE_CBS = list(range(PE_NCB))
PE_CB_IDX = {cb: i for i, cb in enumerate(PE_CBS)}
DVE_CBS = list(range(PE_NCB, CB))


def _build_kernel(ctx, tc):
    nc = tc.nc
    x_d = nc.dram_tensor("x", [BPC, T, D], F32, kind="ExternalInput").ap()
    w1p_d = nc.dram_tensor("w1p", [D, E], BF16, kind="ExternalInput").ap()
    b1p_d = nc.dram_tensor("b1p", [E], F32, kind="ExternalInput").ap()
    w2t_d = nc.dram_tensor("w2t", [I, D], BF16, kind="ExternalInput").ap()
    diag_d = nc.dram_tensor(
        "diag", [P, len(PE_CBS) * K * P], BF16, kind="ExternalInput").ap()
    dwc_d = nc.dram_tensor("dwc", [P, CB * K], F32, kind="ExternalInput").ap()
    dwb_d = nc.dram_tensor("dwbc", [P, CB], F32, kind="ExternalInput").ap()
    alpha_d = nc.dram_tensor("alphac", [P, CB], F32, kind="ExternalInput").ap()
    b2r_d = nc.dram_tensor("b2r", [1, D], BF16, kind="ExternalInput").ap()
    ones_d = nc.dram_tensor("ones", [1, P], BF16, kind="ExternalInput").ap()
    out_d = nc.dram_tensor("out", [BPC, T, D], F32, kind="ExternalOutput").ap()

    const = ctx.enter_context(tc.tile_pool(name="const", bufs=1))
    psum = ctx.enter_context(tc.tile_pool(name="psum", bufs=6, space="PSUM"))

    ident_bf = const.tile([P, P], BF16, tag="ident_bf")
    make_identity(nc, ident_bf[:])

    # ---- constants / weights (all host-prepared) ----
    w1t = [const.tile([P, E], BF16, tag=f"w1t{j}", name=f"w1t{j}")
           for j in range(DCH)]
    for j in range(DCH):
        nc.sync.dma_start(w1t[j][:], w1p_d[j * P:(j + 1) * P, :])
    w2t = [const.tile([P, D], BF16, tag=f"w2t{i}", name=f"w2t{i}")
           for i in range(CB)]
    for i in range(CB):
        nc.sync.dma_start(w2t[i][:], w2t_d[i * P:(i + 1) * P, :])
    b1p = const.tile([P, 2 * CB], F32, tag="b1p")
    nc.sync.dma_start(b1p[:], b1p_d.rearrange("(i p) -> p i", p=P))
    dw_sb = const.tile([P, CB * K], F32, tag="dw_sb")
    nc.sync.dma_start(dw_sb[:], dwc_d)
    dwb_sb = const.tile([P, CB], F32, tag="dwb_sb")
    nc.sync.dma_start(dwb_sb[:], dwb_d)
    alpha_sb = const.tile([P, CB], F32, tag="alpha_sb")
    nc.sync.dma_start(alpha_sb[:], alpha_d)
    b2row = const.tile([1, D], BF16, tag="b2row")
    nc.sync.dma_start(b2row[:], b2r_d)
    ones_bf = const.tile([1, P], BF16, tag="ones_bf")
    nc.sync.dma_start(ones_bf[:], ones_d)
    eps_t = const.tile([P, 1], F32, tag="eps_t")
    nc.vector.memset(eps_t[:], 1e-5)
    diag_sb = const.tile([P, len(PE_CBS) * K * P], BF16, tag="diag")
    nc.sync.dma_start(diag_sb[:], diag_d)

    # ---- pools ----
    xpool = ctx.enter_context(tc.tile_pool(name="xpool", bufs=2))
    scr = ctx.enter_context(tc.tile_pool(name="scr", bufs=2))
    stat = ctx.enter_context(tc.tile_pool(name="stat", bufs=8))
    xnp = ctx.enter_context(tc.tile_pool(name="xnp", bufs=4))
    xntp = ctx.enter_context(tc.tile_pool(name="xntp", bufs=6))
    sw = ctx.enter_context(tc.tile_pool(name="sw", bufs=4))
    strips = ctx.enter_context(tc.tile_pool(name="strips", bufs=8))
    soddp = ctx.enter_context(tc.tile_pool(name="soddp", bufs=4))
    accp = ctx.enter_context(tc.tile_pool(name="accp", bufs=4))
    prodp = ctx.enter_context(tc.tile_pool(name="prodp", bufs=2))
    vtp = ctx.enter_context(tc.tile_pool(name="vtp", bufs=8))
    outp = ctx.enter_context(tc.tile_pool(name="outp", bufs=2))

    strip = {}
    sodd = {}
    vt = {}

    def load_x_panel(b, tp):
        tiles = []
        for tt in range(4):
            t0 = tp * 512 + tt * P
            x_t = xpool.tile([P, D], F32, tag="x", bufs=6,
                             name=f"x_{b}_{tp}_{tt}")
            nc.sync.dma_start(x_t[:], x_d[b, t0:t0 + P, :])
            tiles.append(x_t)
        return tiles

    xq = {}

    def emit_A_panel(b, tp):
        """LN stats + normalize + transpose + GEMM1 + SwiGLU for one panel."""
        if tp + 1 < TP:
            xq[(b, tp + 1)] = load_x_panel(b, tp + 1)
        elif b + 1 < BPC:
            xq[(b + 1, 0)] = load_x_panel(b + 1, 0)
        x_tiles = xq.pop((b, tp))

        ssum4 = stat.tile([P, 4], F32, tag="ssum4")
        ssq4 = stat.tile([P, 4], F32, tag="ssq4")
        for tt in range(4):
            xcp = scr.tile([P, D], BF16, tag="xscr", bufs=2)
            nc.scalar.activation(xcp[:], x_tiles[tt][:], ACTF.Identity,
                                 accum_out=ssum4[:, tt:tt + 1])
            xsq = scr.tile([P, D], BF16, tag="xscr", bufs=2)
            nc.scalar.activation(xsq[:], x_tiles[tt][:], ACTF.Square,
                                 accum_out=ssq4[:, tt:tt + 1])
        negmean4 = stat.tile([P, 4], F32, tag="negmean4")
        nc.vector.tensor_scalar_mul(negmean4[:], ssum4[:], -1.0 / D)
        ex24 = stat.tile([P, 4], F32, tag="ex24")
        nc.vector.tensor_scalar_mul(ex24[:], ssq4[:], 1.0 / D)
        mu24 = stat.tile([P, 4], F32, tag="mu24")
        nc.vector.tensor_mul(mu24[:], negmean4[:], negmean4[:])
        negv4 = stat.tile([P, 4], F32, tag="negv4")
        nc.vector.tensor_sub(negv4[:], mu24[:], ex24[:])
        stdv4 = stat.tile([P, 4], F32, tag="stdv4")
        nc.scalar.activation(stdv4[:], negv4[:], ACTF.Sqrt,
                             scale=-1.0, bias=eps_t[:])
        rstd4 = stat.tile([P, 4], F32, tag="rstd4")
        nc.vector.reciprocal(rstd4[:], stdv4[:])
        negmr4 = stat.tile([P, 4], F32, tag="negmr4")
        nc.vector.tensor_mul(negmr4[:], negmean4[:], rstd4[:])

        xn_tiles = []
        for tt in range(4):
            xn_t = xnp.tile([P, D], BF16, tag="xn")
            nc.scalar.activation(xn_t[:], x_tiles[tt][:], ACTF.Identity,
                                 bias=negmr4[:, tt:tt + 1],
                                 scale=rstd4[:, tt:tt + 1])
            xn_tiles.append(xn_t)
        xnt_p = []
        for j in range(DCH):
            ptr = psum.tile([P, 512], BF16, tag="pst", bufs=2)
            for tt in range(4):
                nc.tensor.transpose(ptr[:, tt * P:(tt + 1) * P],
                                    xn_tiles[tt][:, j * P:(j + 1) * P],
                                    ident_bf[:])
            xt = xntp.tile([P, 512], BF16, tag="xnt", name=f"xnt_{b}_{tp}_{j}")
            nc.scalar.activation(xt[:], ptr[:], ACTF.Copy)
            xnt_p.append(xt)

        for i in range(CB):
            ps_a = psum.tile([P, 512], F32, tag="ps")
            ps_g = psum.tile([P, 512], F32, tag="ps")
            for j in range(DCH):
                nc.tensor.matmul(
                    ps_a[:], w1t[j][:, i * P:(i + 1) * P], xnt_p[j][:],
                    start=(j == 0), stop=(j == DCH - 1))
            for j in range(DCH):
                nc.tensor.matmul(
                    ps_g[:], w1t[j][:, I + i * P:I + (i + 1) * P], xnt_p[j][:],
                    start=(j == 0), stop=(j == DCH - 1))
            s_sb = sw.tile([P, 512], BF16, tag="s_sb")
            nc.scalar.activation(s_sb[:], ps_g[:], ACTF.Silu,
                                 bias=b1p[:, CB + i:CB + i + 1])
            a_sb = sw.tile([P, 512], BF16, tag="a_sb")
            nc.scalar.activation(a_sb[:], ps_a[:], ACTF.Identity,
                                 bias=b1p[:, i:i + 1])
            nc.vector.tensor_mul(
                strip[(b, i)][:, PADL + tp * 512:PADL + (tp + 1) * 512],
                a_sb[:], s_sb[:])

    def alloc_strips(b):
        for cb in range(CB):
            s = strips.tile([P, STRIPW], BF16, tag="strip",
                            name=f"strip_{b}_{cb}")
            nc.gpsimd.memset(s[:, 0:PADL], 0.0)
            nc.gpsimd.memset(s[:, PADL + T:STRIPW], 0.0)
            strip[(b, cb)] = s
            vt[(b, cb)] = vtp.tile([P, T], BF16, tag="vt",
                                   name=f"vt_{b}_{cb}")

    def emit_odd_copy(b, cb):
        so = soddp.tile([P, STRIPW], BF16, tag="sodd", name=f"sodd_{b}_{cb}")
        nc.vector.tensor_copy(so[:, 0:STRIPW - 2],
                              strip[(b, cb)][:, 1:STRIPW - 1])
        sodd[(b, cb)] = so

    def emit_conv_pe(b, cb, tp):
        ps_c = psum.tile([P, 512], F32, tag="ps")
        ci = PE_CB_IDX[cb]
        st = strip[(b, cb)]
        for k in range(K):
            nc.tensor.matmul(
                ps_c[:], diag_sb[:, (ci * K + k) * P:(ci * K + k + 1) * P],
                st[:, tp * 512 + k + 1:tp * 512 + k + 1 + 512],
                start=(k == 0), stop=(k == K - 1))
        nc.scalar.activation(vt[(b, cb)][:, tp * 512:(tp + 1) * 512], ps_c[:],
                             ACTF.Prelu, bias=dwb_sb[:, cb:cb + 1],
                             alpha=alpha_sb[:, cb:cb + 1])

    def win(b, cb, t0, k, L):
        off = t0 + k + 1
        if off % 2 == 0:
            return strip[(b, cb)][:, off:off + L]
        return sodd[(b, cb)][:, off - 1:off - 1 + L]

    def emit_chain(b, cb, t0, L, k0=0, k1=K, acc=None):
        """DVE product+add tree for strip cols [t0,t0+L), taps [k0,k1)."""
        if acc is None:
            acc = accp.tile([P, T], BF16, tag="acc", name=f"acc_{b}_{cb}")
        wcol = lambda k: dw_sb[:, cb * K + k:cb * K + k + 1]
        a = acc[:, t0:t0 + L]
        ks = k0
        if k0 == 0:
            nc.vector.tensor_scalar(a, win(b, cb, t0, 0, L), wcol(0),
                                    dwb_sb[:, cb:cb + 1],
                                    op0=ALU.mult, op1=ALU.add)
            ks = 1
        for k in range(ks, k1):
            pk = prodp.tile([P, 1024], BF16, tag="pk", bufs=3)
            nc.vector.tensor_scalar_mul(pk[:, 0:L], win(b, cb, t0, k, L),
                                        wcol(k))
            nc.vector.tensor_add(a, a, pk[:, 0:L])
        return acc

    def emit_chain_prelu(b, cb, t0, L, acc):
        nc.scalar.activation(vt[(b, cb)][:, t0:t0 + L], acc[:, t0:t0 + L],
                             ACTF.Prelu, alpha=alpha_sb[:, cb:cb + 1])

    def emit_C(b, tp):
        for tt in range(4):
            ps_o = psum.tile([P, D], F32, tag="ps")
            nc.tensor.matmul(ps_o[:], ones_bf[:], b2row[:],
                             start=True, stop=False)
            c0 = tp * 512 + tt * P
            for cb in range(CB):
                nc.tensor.matmul(
                    ps_o[:], vt[(b, cb)][:, c0:c0 + P], w2t[cb][:],
                    start=False, stop=(cb == CB - 1))
            o_sb = outp.tile([P, D], F32, tag="o_sb")
            nc.scalar.activation(o_sb[:], ps_o[:], ACTF.Copy)
            nc.scalar.dma_start(out_d[b, c0:c0 + P, :], o_sb[:])

    # ================= emission =================
    xq[(0, 0)] = load_x_panel(0, 0)
    alloc_strips(0)
    for tp in range(TP):
        emit_A_panel(0, tp)
    for cb in range(4, CB):
        emit_odd_copy(0, cb)

    # conv b0 PE part: cbs 0-3 full + cb4 tp0/tp1
    for cb in (0, 1, 2, 3):
        for tp in range(TP):
            emit_conv_pe(0, cb, tp)
    emit_conv_pe(0, 4, 0)
    emit_conv_pe(0, 4, 1)

    # b0 chains: cb5-7 full strips (two FD1024 column groups), cb4 h2
    acc0 = {}
    acc0[5] = emit_chain(0, 5, 0, 1024)
    emit_chain(0, 5, 1024, 1024, acc=acc0[5])
    alloc_strips(1)
    emit_A_panel(1, 0)
    acc0[6] = emit_chain(0, 6, 0, 1024)
    emit_A_panel(1, 1)
    emit_chain(0, 6, 1024, 1024, acc=acc0[6])
    emit_A_panel(1, 2)
    acc0[7] = emit_chain(0, 7, 0, 1024)
    emit_A_panel(1, 3)
    emit_chain(0, 7, 1024, 1024, acc=acc0[7])
    acc0[4] = emit_chain(0, 4, 1024, 1024)
    for cb in range(4, CB):
        emit_odd_copy(1, cb)
    emit_chain_prelu(0, 4, 1024, 1024, acc0[4])
    for cb in (5, 6, 7):
        emit_chain_prelu(0, cb, 0, T, acc0[cb])
    for tp in range(TP):
        emit_C(0, tp)

    # conv b1 PE part
    for cb in (0, 1, 2, 3):
        for tp in range(TP):
            emit_conv_pe(1, cb, tp)
    emit_conv_pe(1, 4, 0)
    emit_conv_pe(1, 4, 1)
    # b1 chains in column halves; C(1) pipelined per half
    acc1 = {}
    for cb in (5, 6, 7):
        acc1[cb] = emit_chain(1, cb, 0, 1024)
    for cb in (5, 6, 7):
        emit_chain_prelu(1, cb, 0, 1024, acc1[cb])
    emit_C(1, 0)
    emit_C(1, 1)
    acc1[4] = emit_chain(1, 4, 1024, 1024)
    for cb in (5, 6, 7):
        emit_chain(1, cb, 1024, 1024, acc=acc1[cb])
    emit_chain_prelu(1, 4, 1024, 1024, acc1[4])
    for cb in (5, 6, 7):
        emit_chain_prelu(1, cb, 1024, 1024, acc1[cb])
    emit_C(1, 2)
    emit_C(1, 3)


_NC_CACHE = None


def _get_program():
    global _NC_CACHE
    if _NC_CACHE is None:
        nc = bacc.Bacc("TRN2", target_bir_lowering=False, debug=False)
        with tile.TileContext(nc) as tc, ExitStack() as ctx:
            _build_kernel(ctx, tc)
        nc.compile()
        _NC_CACHE = nc
    return _NC_CACHE


def _host_prep(ln_g, ln_b, w1, b1, dw, dwb, alpha, w2, b2):
    bf = ml_dtypes.bfloat16
    w1 = np.asarray(w1, np.float32)
    ln_g = np.asarray(ln_g, np.float32)
    ln_b = np.asarray(ln_b, np.float32)
    dwf = np.asarray(dw, np.float32).reshape(I, K)
    w1p = np.ascontiguousarray((w1 * ln_g[None, :]).T).astype(bf)
    b1p = (np.asarray(b1, np.float32) + w1 @ ln_b).astype(np.float32)
    w2t = np.ascontiguousarray(np.asarray(w2, np.float32).T).astype(bf)
    diag = np.zeros((P, len(PE_CBS) * K * P), np.float32)
    ar = np.arange(P)
    for ci, cb in enumerate(PE_CBS):
        for k in range(K):
            diag[ar, (ci * K + k) * P + ar] = dwf[cb * P:(cb + 1) * P, k]
    diag = diag.astype(bf)
    dwc = np.ascontiguousarray(
        dwf.reshape(CB, P, K).transpose(1, 0, 2).reshape(P, CB * K)
    ).astype(np.float32)
    dwbc = np.ascontiguousarray(
        np.asarray(dwb, np.float32).reshape(CB, P).T).astype(np.float32)
    alphac = np.ascontiguousarray(
        np.asarray(alpha, np.float32).reshape(CB, P).T).astype(np.float32)
    b2r = np.asarray(b2, np.float32)[None, :].astype(bf)
    ones = np.ones((1, P), np.float32).astype(bf)
    return {"w1p": w1p, "b1p": b1p, "w2t": w2t, "diag": diag, "dwc": dwc,
            "dwbc": dwbc, "alphac": alphac, "b2r": b2r, "ones": ones}


def kernel(x, ln_g, ln_b, w1, b1, dw, dwb, alpha, w2, b2, _trace=False):
    nc = _get_program()
    x = np.ascontiguousarray(x, dtype=np.float32)
    shared = _host_prep(ln_g, ln_b, w1, b1, dw, dwb, alpha, w2, b2)
    in_maps = [
        {"x": x[c * BPC:(c + 1) * BPC], **shared} for c in range(NCORES)
    ]
    res = run_bass_kernel_spmd(nc, in_maps, core_ids=list(range(NCORES)),
                               trace=_trace)
    out = np.concatenate([res.results[c]["out"] for c in range(NCORES)], axis=0)
    if _trace:
        kernel.last_results = res
    return out


# revision 18
# speedup vs baseline: 7.0287x; 1.0480x over previous
"""Trainium2 Bass kernel for nn_ConvModule: LN -> 1x1 conv (D->2I) -> SwiGLU
-> depthwise conv (K=31) -> PReLU -> 1x1 conv (I->D).

Sharding: data-parallel over batch, 2 batches per core across 8 cores.

v3 design:
  - Host-side weight preprocessing (w1^T*ln_g bf16, b1'=b1+w1@ln_b, w2^T bf16,
    conv tap diagonals bf16, per-channel tap/alpha/dwb columns).
  - bf16 on-chip data path (DVE 2x/4x perf modes; PE bf16 matmuls).
  - LN stats via ACT accum passes; normalize on ACT (scale/bias [P,1] APs);
    xn^T via PE transposes (bf16) + ACT PSUM->SBUF copies.
  - Depthwise conv split: cbs 0-3 on the PE (31 diagonal matmuls per panel);
    cbs 4-7 on the DVE as product(tensor_scalar 4x) + add(tensor_tensor 2x)
    trees over full strips, with an odd-shifted strip copy so every window
    is 4B-aligned. scalar_tensor_tensor is avoided (no 2x uop).
  - PReLU on ACT (Prelu w/ per-channel alpha AP; dwb fused as bias for PE
    panels, pre-added via the product seed for DVE chains).
  - GEMM2 with v tiles stationary; b2 via a K=1 ones-row matmul.
"""

import sys

sys.path.insert(0, "/opt/trn_rl_repo")

from contextlib import ExitStack

import numpy as np
import ml_dtypes

import concourse.bacc as bacc
import concourse.tile as tile
from concourse import mybir
from concourse.masks import make_identity
from concourse.bass_utils import run_bass_kernel_spmd

B, T, D, I, K = 16, 2048, 512, 1024, 31
NCORES = 8
BPC = B // NCORES  # batches per core
E = 2 * I  # 2048
TP = T // 512  # time panels per batch (4)
CB = I // 128  # channel blocks (8)
DCH = D // 128  # d chunks (4)
PADL = 16  # left pad (even so DVE chain windows stay 4B-aligned)
STRIPW = PADL + T + 16  # 2080
P = 128

F32 = mybir.dt.float32
BF16 = mybir.dt.bfloat16
ALU = mybir.AluOpType
ACTF = mybir.ActivationFunctionType

# cbs 0..PE_NCB-1 on the PE, the rest on DVE chains
PE_NCB = 5
PE_CBS = list(range(PE_NCB))
PE_CB_IDX = {cb: i for i, cb in enumerate(PE_CBS)}
DVE_CBS = list(range(PE_NCB, CB))


def _build_kernel(ctx, tc):
    nc = tc.nc
    x_d = nc.dram_tensor("x", [BPC, T, D], F32, kind="ExternalInput").ap()
    w1p_d = nc.dram_tensor("w1p", [D, E], BF16, kind="ExternalInput").ap()
    b1p_d = nc.dram_tensor("b1p", [E], F32, kind="ExternalInput").ap()
    w2t_d = nc.dram_tensor("w2t", [I, D], BF16, kind="ExternalInput").ap()
    diag_d = nc.dram_tensor(
        "diag", [P, len(PE_CBS) * K * P], BF16, kind="ExternalInput").ap()
    dwc_d = nc.dram_tensor("dwc", [P, CB * K], F32, kind="ExternalInput").ap()
    dwb_d = nc.dram_tensor("dwbc", [P, CB], F32, kind="ExternalInput").ap()
    alpha_d = nc.dram_tensor("alphac", [P, CB], F32, kind="ExternalInput").ap()
    b2r_d = nc.dram_tensor("b2r", [1, D], BF16, kind="ExternalInput").ap()
    ones_d = nc.dram_tensor("ones", [1, P], BF16, kind="ExternalInput").ap()
    out_d = nc.dram_tensor("out", [BPC, T, D], F32, kind="ExternalOutput").ap()

    const = ctx.enter_context(tc.tile_pool(name="const", bufs=1))
    psum = ctx.enter_context(tc.tile_pool(name="psum", bufs=6, space="PSUM"))

    ident_bf = const.tile([P, P], BF16, tag="ident_bf")
    make_identity(nc, ident_bf[:])

    # ---- constants / weights (all host-prepared) ----
    w1t = [const.tile([P, E], BF16, tag=f"w1t{j}", name=f"w1t{j}")
           for j in range(DCH)]
    for j in range(DCH):
        nc.sync.dma_start(w1t[j][:], w1p_d[j * P:(j + 1) * P, :])
    w2t = [const.tile([P, D], BF16, tag=f"w2t{i}", name=f"w2t{i}")
           for i in range(CB)]
    b1p = const.tile([P, 2 * CB], F32, tag="b1p")
    nc.sync.dma_start(b1p[:], b1p_d.rearrange("(i p) -> p i", p=P))
    dw_sb = const.tile([P, CB * K], F32, tag="dw_sb")
    nc.sync.dma_start(dw_sb[:], dwc_d)
    dwb_sb = const.tile([P, CB], F32, tag="dwb_sb")
    nc.sync.dma_start(dwb_sb[:], dwb_d)
    alpha_sb = const.tile([P, CB], F32, tag="alpha_sb")
    nc.sync.dma_start(alpha_sb[:], alpha_d)
    b2row = const.tile([1, D], BF16, tag="b2row")
    nc.sync.dma_start(b2row[:], b2r_d)
    ones_bf = const.tile([1, P], BF16, tag="ones_bf")
    nc.sync.dma_start(ones_bf[:], ones_d)
    eps_t = const.tile([P, 1], F32, tag="eps_t")
    nc.vector.memset(eps_t[:], 1e-5)
    diag_sb = const.tile([P, len(PE_CBS) * K * P], BF16, tag="diag")

    def load_late_consts():
        for i in range(CB):
            nc.sync.dma_start(w2t[i][:], w2t_d[i * P:(i + 1) * P, :])
        nc.sync.dma_start(diag_sb[:], diag_d)

    # ---- pools ----
    xpool = ctx.enter_context(tc.tile_pool(name="xpool", bufs=2))
    scr = ctx.enter_context(tc.tile_pool(name="scr", bufs=2))
    stat = ctx.enter_context(tc.tile_pool(name="stat", bufs=8))
    xnp = ctx.enter_context(tc.tile_pool(name="xnp", bufs=4))
    xntp = ctx.enter_context(tc.tile_pool(name="xntp", bufs=6))
    sw = ctx.enter_context(tc.tile_pool(name="sw", bufs=4))
    strips = ctx.enter_context(tc.tile_pool(name="strips", bufs=8))
    soddp = ctx.enter_context(tc.tile_pool(name="soddp", bufs=4))
    accp = ctx.enter_context(tc.tile_pool(name="accp", bufs=4))
    prodp = ctx.enter_context(tc.tile_pool(name="prodp", bufs=2))
    vtp = ctx.enter_context(tc.tile_pool(name="vtp", bufs=8))
    outp = ctx.enter_context(tc.tile_pool(name="outp", bufs=2))

    strip = {}
    sodd = {}
    vt = {}

    def load_x_panel(b, tp):
        tiles = []
        for tt in range(4):
            t0 = tp * 512 + tt * P
            x_t = xpool.tile([P, D], F32, tag="x", bufs=6,
                             name=f"x_{b}_{tp}_{tt}")
            nc.sync.dma_start(x_t[:], x_d[b, t0:t0 + P, :])
            tiles.append(x_t)
        return tiles

    xq = {}

    def emit_A_panel(b, tp):
        """LN stats + normalize + transpose + GEMM1 + SwiGLU for one panel."""
        if tp + 1 < TP:
            if (b, tp + 1) not in xq:
                xq[(b, tp + 1)] = load_x_panel(b, tp + 1)
        elif b + 1 < BPC:
            xq[(b + 1, 0)] = load_x_panel(b + 1, 0)
        x_tiles = xq.pop((b, tp))

        ssum4 = stat.tile([P, 4], F32, tag="ssum4")
        ssq4 = stat.tile([P, 4], F32, tag="ssq4")
        for tt in range(4):
            xcp = scr.tile([P, D], BF16, tag="xscr", bufs=2)
            nc.scalar.activation(xcp[:], x_tiles[tt][:], ACTF.Identity,
                                 accum_out=ssum4[:, tt:tt + 1])
            xsq = scr.tile([P, D], BF16, tag="xscr", bufs=2)
            nc.scalar.activation(xsq[:], x_tiles[tt][:], ACTF.Square,
                                 accum_out=ssq4[:, tt:tt + 1])
        negmean4 = stat.tile([P, 4], F32, tag="negmean4")
        nc.vector.tensor_scalar_mul(negmean4[:], ssum4[:], -1.0 / D)
        ex24 = stat.tile([P, 4], F32, tag="ex24")
        nc.vector.tensor_scalar_mul(ex24[:], ssq4[:], 1.0 / D)
        mu24 = stat.tile([P, 4], F32, tag="mu24")
        nc.vector.tensor_mul(mu24[:], negmean4[:], negmean4[:])
        negv4 = stat.tile([P, 4], F32, tag="negv4")
        nc.vector.tensor_sub(negv4[:], mu24[:], ex24[:])
        stdv4 = stat.tile([P, 4], F32, tag="stdv4")
        nc.scalar.activation(stdv4[:], negv4[:], ACTF.Sqrt,
                             scale=-1.0, bias=eps_t[:])
        rstd4 = stat.tile([P, 4], F32, tag="rstd4")
        nc.vector.reciprocal(rstd4[:], stdv4[:])
        negmr4 = stat.tile([P, 4], F32, tag="negmr4")
        nc.vector.tensor_mul(negmr4[:], negmean4[:], rstd4[:])

        xn_tiles = []
        for tt in range(4):
            xn_t = xnp.tile([P, D], BF16, tag="xn")
            nc.scalar.activation(xn_t[:], x_tiles[tt][:], ACTF.Identity,
                                 bias=negmr4[:, tt:tt + 1],
                                 scale=rstd4[:, tt:tt + 1])
            xn_tiles.append(xn_t)
        xnt_p = []
        for j in range(DCH):
            ptr = psum.tile([P, 512], BF16, tag="pst", bufs=2)
            for tt in range(4):
                nc.tensor.transpose(ptr[:, tt * P:(tt + 1) * P],
                                    xn_tiles[tt][:, j * P:(j + 1) * P],
                                    ident_bf[:])
            xt = xntp.tile([P, 512], BF16, tag="xnt", name=f"xnt_{b}_{tp}_{j}")
            nc.scalar.activation(xt[:], ptr[:], ACTF.Copy)
            xnt_p.append(xt)

        for i in range(CB):
            ps_a = psum.tile([P, 512], F32, tag="ps")
            ps_g = psum.tile([P, 512], F32, tag="ps")
            for j in range(DCH):
                nc.tensor.matmul(
                    ps_a[:], w1t[j][:, i * P:(i + 1) * P], xnt_p[j][:],
                    start=(j == 0), stop=(j == DCH - 1))
            for j in range(DCH):
                nc.tensor.matmul(
                    ps_g[:], w1t[j][:, I + i * P:I + (i + 1) * P], xnt_p[j][:],
                    start=(j == 0), stop=(j == DCH - 1))
            s_sb = sw.tile([P, 512], BF16, tag="s_sb")
            nc.scalar.activation(s_sb[:], ps_g[:], ACTF.Silu,
                                 bias=b1p[:, CB + i:CB + i + 1])
            nc.vector.scalar_tensor_tensor(
                strip[(b, i)][:, PADL + tp * 512:PADL + (tp + 1) * 512],
                ps_a[:], b1p[:, i:i + 1], s_sb[:],
                op0=ALU.add, op1=ALU.mult)

    def alloc_strips(b):
        for cb in range(CB):
            s = strips.tile([P, STRIPW], BF16, tag="strip",
                            name=f"strip_{b}_{cb}")
            nc.gpsimd.memset(s[:, 0:PADL], 0.0)
            nc.gpsimd.memset(s[:, PADL + T:STRIPW], 0.0)
            strip[(b, cb)] = s
            vt[(b, cb)] = vtp.tile([P, T], BF16, tag="vt",
                                   name=f"vt_{b}_{cb}")

    def emit_odd_copy(b, cb):
        so = soddp.tile([P, STRIPW], BF16, tag="sodd", name=f"sodd_{b}_{cb}")
        nc.vector.tensor_copy(so[:, 0:STRIPW - 2],
                              strip[(b, cb)][:, 1:STRIPW - 1])
        sodd[(b, cb)] = so

    def emit_conv_pe(b, cb, tp):
        ps_c = psum.tile([P, 512], F32, tag="ps")
        ci = PE_CB_IDX[cb]
        st = strip[(b, cb)]
        for k in range(K):
            nc.tensor.matmul(
                ps_c[:], diag_sb[:, (ci * K + k) * P:(ci * K + k + 1) * P],
                st[:, tp * 512 + k + 1:tp * 512 + k + 1 + 512],
                start=(k == 0), stop=(k == K - 1))
        nc.scalar.activation(vt[(b, cb)][:, tp * 512:(tp + 1) * 512], ps_c[:],
                             ACTF.Prelu, bias=dwb_sb[:, cb:cb + 1],
                             alpha=alpha_sb[:, cb:cb + 1])

    def win(b, cb, t0, k, L):
        off = t0 + k + 1
        if off % 2 == 0:
            return strip[(b, cb)][:, off:off + L]
        return sodd[(b, cb)][:, off - 1:off - 1 + L]

    def emit_chain(b, cb, t0, L, k0=0, k1=K, acc=None, prod_act=False):
        """product+add tree for strip cols [t0,t0+L), taps [k0,k1).
        Adds on DVE; products on DVE, or on ACT when prod_act."""
        if acc is None:
            acc = accp.tile([P, T], BF16, tag="acc", name=f"acc_{b}_{cb}")
        wcol = lambda k: dw_sb[:, cb * K + k:cb * K + k + 1]
        a = acc[:, t0:t0 + L]
        ks = k0
        if k0 == 0:
            nc.vector.tensor_scalar(a, win(b, cb, t0, 0, L), wcol(0),
                                    dwb_sb[:, cb:cb + 1],
                                    op0=ALU.mult, op1=ALU.add)
            ks = 1
        for k in range(ks, k1):
            pk = prodp.tile([P, 1024], BF16, tag="pk", bufs=3)
            if prod_act:
                nc.scalar.activation(pk[:, 0:L], win(b, cb, t0, k, L),
                                     ACTF.Copy, scale=wcol(k))
            else:
                nc.vector.tensor_scalar_mul(pk[:, 0:L], win(b, cb, t0, k, L),
                                            wcol(k))
            nc.vector.tensor_add(a, a, pk[:, 0:L])
        return acc

    def emit_chain_prelu(b, cb, t0, L, acc):
        nc.scalar.activation(vt[(b, cb)][:, t0:t0 + L], acc[:, t0:t0 + L],
                             ACTF.Prelu, alpha=alpha_sb[:, cb:cb + 1])

    def emit_C(b, tp):
        for tt in range(4):
            ps_o = psum.tile([P, D], F32, tag="ps")
            nc.tensor.matmul(ps_o[:], ones_bf[:], b2row[:],
                             start=True, stop=False)
            c0 = tp * 512 + tt * P
            for cb in range(CB):
                nc.tensor.matmul(
                    ps_o[:], vt[(b, cb)][:, c0:c0 + P], w2t[cb][:],
                    start=False, stop=(cb == CB - 1))
            o_sb = outp.tile([P, D], F32, tag="o_sb")
            nc.scalar.activation(o_sb[:], ps_o[:], ACTF.Copy)
            nc.scalar.dma_start(out_d[b, c0:c0 + P, :], o_sb[:])

    # ================= emission =================
    xq[(0, 0)] = load_x_panel(0, 0)
    alloc_strips(0)
    xq[(0, 1)] = load_x_panel(0, 1)
    emit_A_panel(0, 0)
    load_late_consts()
    for tp in range(1, TP):
        emit_A_panel(0, tp)
    for cb in range(4, CB):
        emit_odd_copy(0, cb)

    # conv b0 PE part: cbs 0-3 full + cb4 tp0/tp1
    for cb in (0, 1, 2, 3):
        for tp in range(TP):
            emit_conv_pe(0, cb, tp)
    emit_conv_pe(0, 4, 0)
    emit_conv_pe(0, 4, 1)
    emit_conv_pe(0, 4, 2)

    # b0 chains: cb5-7 full strips (two FD1024 column groups), cb4 h2
    acc0 = {}
    acc0[5] = emit_chain(0, 5, 0, 1024)
    emit_chain(0, 5, 1024, 1024, acc=acc0[5])
    alloc_strips(1)
    emit_A_panel(1, 0)
    acc0[6] = emit_chain(0, 6, 0, 1024)
    emit_A_panel(1, 1)
    emit_chain(0, 6, 1024, 1024, acc=acc0[6])
    emit_A_panel(1, 2)
    acc0[7] = emit_chain(0, 7, 0, 1024)
    emit_A_panel(1, 3)
    emit_chain(0, 7, 1024, 1024, acc=acc0[7])
    acc0[4] = emit_chain(0, 4, 1536, 512)
    for cb in range(4, CB):
        emit_odd_copy(1, cb)
    emit_chain_prelu(0, 4, 1536, 512, acc0[4])
    for cb in (5, 6, 7):
        emit_chain_prelu(0, cb, 0, T, acc0[cb])
    for tp in range(TP):
        emit_C(0, tp)

    # conv b1 PE part
    for cb in (0, 1, 2, 3):
        for tp in range(TP):
            emit_conv_pe(1, cb, tp)
    emit_conv_pe(1, 4, 0)
    emit_conv_pe(1, 4, 1)
    emit_conv_pe(1, 4, 2)
    # b1 chains in column halves; C(1) pipelined per half
    acc1 = {}
    for cb in (5, 6, 7):
        acc1[cb] = emit_chain(1, cb, 0, 1024)
    for cb in (5, 6, 7):
        emit_chain_prelu(1, cb, 0, 1024, acc1[cb])
    emit_C(1, 0)
    emit_C(1, 1)
    acc1[4] = emit_chain(1, 4, 1536, 512)
    for cb in (5, 6, 7):
        emit_chain(1, cb, 1024, 1024, acc=acc1[cb],
                   prod_act=(cb in (6, 7)))
    emit_chain_prelu(1, 4, 1536, 512, acc1[4])
    for cb in (5, 6, 7):
        emit_chain_prelu(1, cb, 1024, 1024, acc1[cb])
    emit_C(1, 2)
    emit_C(1, 3)


_NC_CACHE = None


def _get_program():
    global _NC_CACHE
    if _NC_CACHE is None:
        nc = bacc.Bacc("TRN2", target_bir_lowering=False, debug=False)
        with tile.TileContext(nc) as tc, ExitStack() as ctx:
            _build_kernel(ctx, tc)
        nc.compile()
        _NC_CACHE = nc
    return _NC_CACHE


def _host_prep(ln_g, ln_b, w1, b1, dw, dwb, alpha, w2, b2):
    bf = ml_dtypes.bfloat16
    w1 = np.asarray(w1, np.float32)
    ln_g = np.asarray(ln_g, np.float32)
    ln_b = np.asarray(ln_b, np.float32)
    dwf = np.asarray(dw, np.float32).reshape(I, K)
    w1p = np.ascontiguousarray((w1 * ln_g[None, :]).T).astype(bf)
    b1p = (np.asarray(b1, np.float32) + w1 @ ln_b).astype(np.float32)
    w2t = np.ascontiguousarray(np.asarray(w2, np.float32).T).astype(bf)
    diag = np.zeros((P, len(PE_CBS) * K * P), np.float32)
    ar = np.arange(P)
    for ci, cb in enumerate(PE_CBS):
        for k in range(K):
            diag[ar, (ci * K + k) * P + ar] = dwf[cb * P:(cb + 1) * P, k]
    diag = diag.astype(bf)
    dwc = np.ascontiguousarray(
        dwf.reshape(CB, P, K).transpose(1, 0, 2).reshape(P, CB * K)
    ).astype(np.float32)
    dwbc = np.ascontiguousarray(
        np.asarray(dwb, np.float32).reshape(CB, P).T).astype(np.float32)
    alphac = np.ascontiguousarray(
        np.asarray(alpha, np.float32).reshape(CB, P).T).astype(np.float32)
    b2r = np.asarray(b2, np.float32)[None, :].astype(bf)
    ones = np.ones((1, P), np.float32).astype(bf)
    return {"w1p": w1p, "b1p": b1p, "w2t": w2t, "diag": diag, "dwc": dwc,
            "dwbc": dwbc, "alphac": alphac, "b2r": b2r, "ones": ones}


def kernel(x, ln_g, ln_b, w1, b1, dw, dwb, alpha, w2, b2, _trace=False):
    nc = _get_program()
    x = np.ascontiguousarray(x, dtype=np.float32)
    shared = _host_prep(ln_g, ln_b, w1, b1, dw, dwb, alpha, w2, b2)
    in_maps = [
        {"x": x[c * BPC:(c + 1) * BPC], **shared} for c in range(NCORES)
    ]
    res = run_bass_kernel_spmd(nc, in_maps, core_ids=list(range(NCORES)),
                               trace=_trace)
    out = np.concatenate([res.results[c]["out"] for c in range(NCORES)], axis=0)
    if _trace:
        kernel.last_results = res
    return out


# revision 19
# speedup vs baseline: 7.6582x; 1.0896x over previous
"""Trainium2 Bass kernel for nn_ConvModule: LN -> 1x1 conv (D->2I) -> SwiGLU
-> depthwise conv (K=31) -> PReLU -> 1x1 conv (I->D).

Sharding: data-parallel over batch, 2 batches per core across 8 cores.

v3 design:
  - Host-side weight preprocessing (w1^T*ln_g bf16, b1'=b1+w1@ln_b, w2^T bf16,
    conv tap diagonals bf16, per-channel tap/alpha/dwb columns).
  - bf16 on-chip data path (DVE 2x/4x perf modes; PE bf16 matmuls).
  - LN stats via ACT accum passes; normalize on ACT (scale/bias [P,1] APs);
    xn^T via PE transposes (bf16) + ACT PSUM->SBUF copies.
  - Depthwise conv split: cbs 0-3 on the PE (31 diagonal matmuls per panel);
    cbs 4-7 on the DVE as product(tensor_scalar 4x) + add(tensor_tensor 2x)
    trees over full strips, with an odd-shifted strip copy so every window
    is 4B-aligned. scalar_tensor_tensor is avoided (no 2x uop).
  - PReLU on ACT (Prelu w/ per-channel alpha AP; dwb fused as bias for PE
    panels, pre-added via the product seed for DVE chains).
  - GEMM2 with v tiles stationary; b2 via a K=1 ones-row matmul.
"""

import sys

sys.path.insert(0, "/opt/trn_rl_repo")

from contextlib import ExitStack

import numpy as np
import ml_dtypes

import concourse.bacc as bacc
import concourse.tile as tile
from concourse import mybir
from concourse.masks import make_identity
from concourse.bass_utils import run_bass_kernel_spmd

B, T, D, I, K = 16, 2048, 512, 1024, 31
NCORES = 8
BPC = B // NCORES  # batches per core
E = 2 * I  # 2048
TP = T // 512  # time panels per batch (4)
CB = I // 128  # channel blocks (8)
DCH = D // 128  # d chunks (4)
PADL = 16  # left pad (even so DVE chain windows stay 4B-aligned)
STRIPW = PADL + T + 16  # 2080
P = 128

F32 = mybir.dt.float32
BF16 = mybir.dt.bfloat16
ALU = mybir.AluOpType
ACTF = mybir.ActivationFunctionType

# cbs 0..PE_NCB-1 on the PE, the rest on DVE chains
PE_NCB = 5
PE_CBS = list(range(PE_NCB))
PE_CB_IDX = {cb: i for i, cb in enumerate(PE_CBS)}
DVE_CBS = list(range(PE_NCB, CB))


def _build_kernel(ctx, tc):
    nc = tc.nc
    x_d = nc.dram_tensor("x", [BPC, T, D], F32, kind="ExternalInput").ap()
    w1p_d = nc.dram_tensor("w1p", [D, E], BF16, kind="ExternalInput").ap()
    b1p_d = nc.dram_tensor("b1p", [E], F32, kind="ExternalInput").ap()
    w2t_d = nc.dram_tensor("w2t", [I, D], BF16, kind="ExternalInput").ap()
    diag_d = nc.dram_tensor(
        "diag", [P, len(PE_CBS) * K * P], BF16, kind="ExternalInput").ap()
    dwc_d = nc.dram_tensor("dwc", [P, CB * K], F32, kind="ExternalInput").ap()
    dwb_d = nc.dram_tensor("dwbc", [P, CB], F32, kind="ExternalInput").ap()
    alpha_d = nc.dram_tensor("alphac", [P, CB], F32, kind="ExternalInput").ap()
    b2r_d = nc.dram_tensor("b2r", [1, D], BF16, kind="ExternalInput").ap()
    ones_d = nc.dram_tensor("ones", [1, P], BF16, kind="ExternalInput").ap()
    out_d = nc.dram_tensor("out", [BPC, T, D], F32, kind="ExternalOutput").ap()

    const = ctx.enter_context(tc.tile_pool(name="const", bufs=1))
    psum = ctx.enter_context(tc.tile_pool(name="psum", bufs=6, space="PSUM"))

    ident_bf = const.tile([P, P], BF16, tag="ident_bf")
    make_identity(nc, ident_bf[:])

    # ---- constants / weights (all host-prepared) ----
    w1t = [const.tile([P, E], BF16, tag=f"w1t{j}", name=f"w1t{j}")
           for j in range(DCH)]
    for j in range(DCH):
        nc.sync.dma_start(w1t[j][:], w1p_d[j * P:(j + 1) * P, :])
    w2t = [const.tile([P, D], BF16, tag=f"w2t{i}", name=f"w2t{i}")
           for i in range(CB)]
    b1p = const.tile([P, 2 * CB], F32, tag="b1p")
    nc.sync.dma_start(b1p[:], b1p_d.rearrange("(i p) -> p i", p=P))
    dw_sb = const.tile([P, CB * K], F32, tag="dw_sb")
    nc.sync.dma_start(dw_sb[:], dwc_d)
    dwb_sb = const.tile([P, CB], F32, tag="dwb_sb")
    nc.sync.dma_start(dwb_sb[:], dwb_d)
    alpha_sb = const.tile([P, CB], F32, tag="alpha_sb")
    nc.sync.dma_start(alpha_sb[:], alpha_d)
    b2row = const.tile([1, D], BF16, tag="b2row")
    nc.sync.dma_start(b2row[:], b2r_d)
    ones_bf = const.tile([1, P], BF16, tag="ones_bf")
    nc.sync.dma_start(ones_bf[:], ones_d)
    eps_t = const.tile([P, 1], F32, tag="eps_t")
    nc.vector.memset(eps_t[:], 1e-5)
    diag_sb = const.tile([P, len(PE_CBS) * K * P], BF16, tag="diag")

    def load_late_consts():
        for i in range(CB):
            nc.sync.dma_start(w2t[i][:], w2t_d[i * P:(i + 1) * P, :])
        nc.sync.dma_start(diag_sb[:], diag_d)

    # ---- pools ----
    xpool = ctx.enter_context(tc.tile_pool(name="xpool", bufs=2))
    scr = ctx.enter_context(tc.tile_pool(name="scr", bufs=2))
    stat = ctx.enter_context(tc.tile_pool(name="stat", bufs=8))
    xnp = ctx.enter_context(tc.tile_pool(name="xnp", bufs=4))
    xntp = ctx.enter_context(tc.tile_pool(name="xntp", bufs=6))
    sw = ctx.enter_context(tc.tile_pool(name="sw", bufs=4))
    strips = ctx.enter_context(tc.tile_pool(name="strips", bufs=8))
    soddp = ctx.enter_context(tc.tile_pool(name="soddp", bufs=3))
    accp = ctx.enter_context(tc.tile_pool(name="accp", bufs=4))
    prodp = ctx.enter_context(tc.tile_pool(name="prodp", bufs=2))
    vtp = ctx.enter_context(tc.tile_pool(name="vtp", bufs=8))
    outp = ctx.enter_context(tc.tile_pool(name="outp", bufs=2))

    strip = {}
    sodd = {}
    vt = {}

    def load_x_panel(b, tp):
        tiles = []
        for tt in range(4):
            t0 = tp * 512 + tt * P
            x_t = xpool.tile([P, D], F32, tag="x", bufs=6,
                             name=f"x_{b}_{tp}_{tt}")
            nc.sync.dma_start(x_t[:], x_d[b, t0:t0 + P, :])
            tiles.append(x_t)
        return tiles

    xq = {}

    def emit_A_panel(b, tp):
        """LN stats + normalize + transpose + GEMM1 + SwiGLU for one panel."""
        if tp + 1 < TP:
            if (b, tp + 1) not in xq:
                xq[(b, tp + 1)] = load_x_panel(b, tp + 1)
        elif b + 1 < BPC:
            xq[(b + 1, 0)] = load_x_panel(b + 1, 0)
        x_tiles = xq.pop((b, tp))

        ssum4 = stat.tile([P, 4], F32, tag="ssum4")
        ssq4 = stat.tile([P, 4], F32, tag="ssq4")
        for tt in range(4):
            xcp = scr.tile([P, D], BF16, tag="xscr", bufs=2)
            nc.scalar.activation(xcp[:], x_tiles[tt][:], ACTF.Identity,
                                 accum_out=ssum4[:, tt:tt + 1])
            xsq = scr.tile([P, D], BF16, tag="xscr", bufs=2)
            nc.scalar.activation(xsq[:], x_tiles[tt][:], ACTF.Square,
                                 accum_out=ssq4[:, tt:tt + 1])
        negmean4 = stat.tile([P, 4], F32, tag="negmean4")
        nc.vector.tensor_scalar_mul(negmean4[:], ssum4[:], -1.0 / D)
        ex24 = stat.tile([P, 4], F32, tag="ex24")
        nc.vector.tensor_scalar_mul(ex24[:], ssq4[:], 1.0 / D)
        mu24 = stat.tile([P, 4], F32, tag="mu24")
        nc.vector.tensor_mul(mu24[:], negmean4[:], negmean4[:])
        negv4 = stat.tile([P, 4], F32, tag="negv4")
        nc.vector.tensor_sub(negv4[:], mu24[:], ex24[:])
        stdv4 = stat.tile([P, 4], F32, tag="stdv4")
        nc.scalar.activation(stdv4[:], negv4[:], ACTF.Sqrt,
                             scale=-1.0, bias=eps_t[:])
        rstd4 = stat.tile([P, 4], F32, tag="rstd4")
        nc.vector.reciprocal(rstd4[:], stdv4[:])
        negmr4 = stat.tile([P, 4], F32, tag="negmr4")
        nc.vector.tensor_mul(negmr4[:], negmean4[:], rstd4[:])

        xn_tiles = []
        for tt in range(4):
            xn_t = xnp.tile([P, D], BF16, tag="xn")
            nc.scalar.activation(xn_t[:], x_tiles[tt][:], ACTF.Identity,
                                 bias=negmr4[:, tt:tt + 1],
                                 scale=rstd4[:, tt:tt + 1])
            xn_tiles.append(xn_t)
        xnt_p = []
        for j in range(DCH):
            ptr = psum.tile([P, 512], BF16, tag="pst", bufs=2)
            for tt in range(4):
                nc.tensor.transpose(ptr[:, tt * P:(tt + 1) * P],
                                    xn_tiles[tt][:, j * P:(j + 1) * P],
                                    ident_bf[:])
            xt = xntp.tile([P, 512], BF16, tag="xnt", name=f"xnt_{b}_{tp}_{j}")
            nc.vector.tensor_copy(xt[:], ptr[:])
            xnt_p.append(xt)

        for i in range(CB):
            ps_a = psum.tile([P, 512], F32, tag="ps")
            ps_g = psum.tile([P, 512], F32, tag="ps")
            for j in range(DCH):
                nc.tensor.matmul(
                    ps_a[:], w1t[j][:, i * P:(i + 1) * P], xnt_p[j][:],
                    start=(j == 0), stop=(j == DCH - 1))
            for j in range(DCH):
                nc.tensor.matmul(
                    ps_g[:], w1t[j][:, I + i * P:I + (i + 1) * P], xnt_p[j][:],
                    start=(j == 0), stop=(j == DCH - 1))
            s_sb = sw.tile([P, 512], BF16, tag="s_sb")
            nc.scalar.activation(s_sb[:], ps_g[:], ACTF.Silu,
                                 bias=b1p[:, CB + i:CB + i + 1])
            nc.vector.scalar_tensor_tensor(
                strip[(b, i)][:, PADL + tp * 512:PADL + (tp + 1) * 512],
                ps_a[:], b1p[:, i:i + 1], s_sb[:],
                op0=ALU.add, op1=ALU.mult)

    def alloc_strips(b):
        for cb in range(CB):
            s = strips.tile([P, STRIPW], BF16, tag="strip",
                            name=f"strip_{b}_{cb}")
            nc.gpsimd.memset(s[:, 0:PADL], 0.0)
            nc.gpsimd.memset(s[:, PADL + T:STRIPW], 0.0)
            strip[(b, cb)] = s
            vt[(b, cb)] = vtp.tile([P, T], BF16, tag="vt",
                                   name=f"vt_{b}_{cb}")

    def emit_odd_copy(b, cb):
        so = soddp.tile([P, STRIPW], BF16, tag="sodd", name=f"sodd_{b}_{cb}")
        nc.vector.tensor_copy(so[:, 0:STRIPW - 2],
                              strip[(b, cb)][:, 1:STRIPW - 1])
        sodd[(b, cb)] = so

    def emit_conv_pe(b, cb, tp):
        ps_c = psum.tile([P, 512], F32, tag="ps")
        ci = PE_CB_IDX[cb]
        st = strip[(b, cb)]
        for k in range(K):
            nc.tensor.matmul(
                ps_c[:], diag_sb[:, (ci * K + k) * P:(ci * K + k + 1) * P],
                st[:, tp * 512 + k + 1:tp * 512 + k + 1 + 512],
                start=(k == 0), stop=(k == K - 1))
        nc.scalar.activation(vt[(b, cb)][:, tp * 512:(tp + 1) * 512], ps_c[:],
                             ACTF.Prelu, bias=dwb_sb[:, cb:cb + 1],
                             alpha=alpha_sb[:, cb:cb + 1])

    def win(b, cb, t0, k, L):
        off = t0 + k + 1
        if off % 2 == 0:
            return strip[(b, cb)][:, off:off + L]
        return sodd[(b, cb)][:, off - 1:off - 1 + L]

    def emit_chain(b, cb, t0, L, k0=0, k1=K, acc=None, prod_act=False):
        """product+add tree for strip cols [t0,t0+L), taps [k0,k1).
        Adds on DVE; products on DVE, or on ACT when prod_act."""
        if acc is None:
            acc = accp.tile([P, T], BF16, tag="acc", name=f"acc_{b}_{cb}")
        wcol = lambda k: dw_sb[:, cb * K + k:cb * K + k + 1]
        a = acc[:, t0:t0 + L]
        ks = k0
        if k0 == 0:
            nc.vector.tensor_scalar(a, win(b, cb, t0, 0, L), wcol(0),
                                    dwb_sb[:, cb:cb + 1],
                                    op0=ALU.mult, op1=ALU.add)
            ks = 1
        for k in range(ks, k1):
            pk = prodp.tile([P, 1024], BF16, tag="pk", bufs=3)
            if prod_act:
                nc.scalar.activation(pk[:, 0:L], win(b, cb, t0, k, L),
                                     ACTF.Copy, scale=wcol(k))
            else:
                nc.vector.tensor_scalar_mul(pk[:, 0:L], win(b, cb, t0, k, L),
                                            wcol(k))
            nc.vector.tensor_add(a, a, pk[:, 0:L])
        return acc

    def emit_chain_prelu(b, cb, t0, L, acc):
        nc.scalar.activation(vt[(b, cb)][:, t0:t0 + L], acc[:, t0:t0 + L],
                             ACTF.Prelu, alpha=alpha_sb[:, cb:cb + 1])

    def emit_C(b, tp):
        for tt in range(4):
            ps_o = psum.tile([P, D], F32, tag="ps")
            nc.tensor.matmul(ps_o[:], ones_bf[:], b2row[:],
                             start=True, stop=False)
            c0 = tp * 512 + tt * P
            for cb in range(CB):
                nc.tensor.matmul(
                    ps_o[:], vt[(b, cb)][:, c0:c0 + P], w2t[cb][:],
                    start=False, stop=(cb == CB - 1))
            o_sb = outp.tile([P, D], F32, tag="o_sb")
            nc.scalar.activation(o_sb[:], ps_o[:], ACTF.Copy)
            nc.scalar.dma_start(out_d[b, c0:c0 + P, :], o_sb[:])

    # ================= emission =================
    xq[(0, 0)] = load_x_panel(0, 0)
    alloc_strips(0)
    xq[(0, 1)] = load_x_panel(0, 1)
    emit_A_panel(0, 0)
    load_late_consts()
    for tp in range(1, TP):
        emit_A_panel(0, tp)
    for cb in (5, 6, 7):
        emit_odd_copy(0, cb)

    # conv b0 PE part: cbs 0-3 full + cb4 tp0/tp1
    for cb in (0, 1, 2, 3):
        for tp in range(TP):
            emit_conv_pe(0, cb, tp)
    for tp in range(TP):
        emit_conv_pe(0, 4, tp)

    # b0 chains: cb5-7 full strips (two FD1024 column groups), cb4 h2
    acc0 = {}
    acc0[5] = emit_chain(0, 5, 0, 1024)
    emit_chain(0, 5, 1024, 1024, acc=acc0[5])
    alloc_strips(1)
    emit_A_panel(1, 0)
    acc0[6] = emit_chain(0, 6, 0, 1024)
    emit_A_panel(1, 1)
    emit_chain(0, 6, 1024, 1024, acc=acc0[6])
    emit_A_panel(1, 2)
    acc0[7] = emit_chain(0, 7, 0, 1024)
    emit_A_panel(1, 3)
    emit_chain(0, 7, 1024, 1024, acc=acc0[7])
    for cb in (5, 6, 7):
        emit_odd_copy(1, cb)
    for cb in (5, 6, 7):
        emit_chain_prelu(0, cb, 0, T, acc0[cb])
    for tp in range(TP):
        emit_C(0, tp)

    # conv b1 PE part
    for cb in (0, 1, 2, 3):
        for tp in range(TP):
            emit_conv_pe(1, cb, tp)
    for tp in range(TP):
        emit_conv_pe(1, 4, tp)
    # b1 chains in column halves; C(1) pipelined per half
    acc1 = {}
    for cb in (5, 6, 7):
        acc1[cb] = emit_chain(1, cb, 0, 1024)
    for cb in (5, 6, 7):
        emit_chain_prelu(1, cb, 0, 1024, acc1[cb])
    emit_C(1, 0)
    emit_C(1, 1)
    for cb in (5, 6, 7):
        emit_chain(1, cb, 1024, 1024, acc=acc1[cb],
                   prod_act=(cb == 7))
    for cb in (5, 6, 7):
        emit_chain_prelu(1, cb, 1024, 1024, acc1[cb])
    emit_C(1, 2)
    emit_C(1, 3)


_NC_CACHE = None


def _get_program():
    global _NC_CACHE
    if _NC_CACHE is None:
        nc = bacc.Bacc("TRN2", target_bir_lowering=False, debug=False)
        with tile.TileContext(nc) as tc, ExitStack() as ctx:
            _build_kernel(ctx, tc)
        nc.compile()
        _NC_CACHE = nc
    return _NC_CACHE


def _host_prep(ln_g, ln_b, w1, b1, dw, dwb, alpha, w2, b2):
    bf = ml_dtypes.bfloat16
    w1 = np.asarray(w1, np.float32)
    ln_g = np.asarray(ln_g, np.float32)
    ln_b = np.asarray(ln_b, np.float32)
    dwf = np.asarray(dw, np.float32).reshape(I, K)
    w1p = np.ascontiguousarray((w1 * ln_g[None, :]).T).astype(bf)
    b1p = (np.asarray(b1, np.float32) + w1 @ ln_b).astype(np.float32)
    w2t = np.ascontiguousarray(np.asarray(w2, np.float32).T).astype(bf)
    diag = np.zeros((P, len(PE_CBS) * K * P), np.float32)
    ar = np.arange(P)
    for ci, cb in enumerate(PE_CBS):
        for k in range(K):
            diag[ar, (ci * K + k) * P + ar] = dwf[cb * P:(cb + 1) * P, k]
    diag = diag.astype(bf)
    dwc = np.ascontiguousarray(
        dwf.reshape(CB, P, K).transpose(1, 0, 2).reshape(P, CB * K)
    ).astype(np.float32)
    dwbc = np.ascontiguousarray(
        np.asarray(dwb, np.float32).reshape(CB, P).T).astype(np.float32)
    alphac = np.ascontiguousarray(
        np.asarray(alpha, np.float32).reshape(CB, P).T).astype(np.float32)
    b2r = np.asarray(b2, np.float32)[None, :].astype(bf)
    ones = np.ones((1, P), np.float32).astype(bf)
    return {"w1p": w1p, "b1p": b1p, "w2t": w2t, "diag": diag, "dwc": dwc,
            "dwbc": dwbc, "alphac": alphac, "b2r": b2r, "ones": ones}


def kernel(x, ln_g, ln_b, w1, b1, dw, dwb, alpha, w2, b2, _trace=False):
    nc = _get_program()
    x = np.ascontiguousarray(x, dtype=np.float32)
    shared = _host_prep(ln_g, ln_b, w1, b1, dw, dwb, alpha, w2, b2)
    in_maps = [
        {"x": x[c * BPC:(c + 1) * BPC], **shared} for c in range(NCORES)
    ]
    res = run_bass_kernel_spmd(nc, in_maps, core_ids=list(range(NCORES)),
                               trace=_trace)
    out = np.concatenate([res.results[c]["out"] for c in range(NCORES)], axis=0)
    if _trace:
        kernel.last_results = res
    return out
